# revision 4
# baseline (speedup 1.0000x reference)
"""v14: v10 + row-split loads so the first copy starts ~1.4us earlier.

SDMA engine 15 has a fixed ~18% bandwidth deficit on stores and serves
1/16 of the bytes (2.36 MB at 22 GB/s = 107 us); it is continuously
busy from the first store to the end, so total time = store_start +
107us + teardown.  This version minimizes store_start:
  - x load on sync ring and y load on scalar ring concurrently,
  - the j=0 stores are sliced per-i (3 stores each, gated on one DVE
    copy apiece) so the first store issues ~2.2us after the load lands,
  - j=1,2 keep single merged stacked stores (48KB descriptors).
"""

import os
import sys

import numpy as np

try:
    import concourse  # noqa: F401
except ImportError:
    for p in ("/root/.axon_site", "/root/.axon_site/_ro/trn_rl_repo",
              "/root/.axon_site/_ro/pypackages", "/opt/trn_rl_repo"):
        if os.path.isdir(p) and p not in sys.path:
            sys.path.append(p)

import concourse.bass as bass
import concourse.mybir as mybir
from concourse.bass_utils import run_bass_kernel_spmd

N_CORES = 8
B, C, H, W = 2, 64, 256, 256
F = 3
ROWS = H // N_CORES  # 32
PATCH = ROWS * W
NBUF = 3

_cache = {}


def _f32_to_bf16_u16(a: np.ndarray) -> np.ndarray:
    u = np.ascontiguousarray(a, dtype=np.float32).view(np.uint32)
    r = (u + 0x7FFF + ((u >> 16) & 1)) >> 16
    return r.astype(np.uint16)


def _bf16_u16_to_f32(u16: np.ndarray) -> np.ndarray:
    return (u16.astype(np.uint32) << 16).view(np.float32)


def _build_nc(d: int) -> bass.Bass:
    PR = ROWS + 2 * d
    PW = W + 2 * d
    bf16 = mybir.dt.bfloat16

    nc = bass.Bass("TRN2", dynamic_dma_scratch_size=2048)
    xs = nc.dram_tensor("xs", [B * C, PR, PW], bf16, kind="ExternalInput")
    ys = nc.dram_tensor("ys", [B * C, PR, PW], bf16, kind="ExternalInput")
    ox = nc.dram_tensor("ox", [B * C, F * F, PATCH], bf16, kind="ExternalOutput")
    oy = nc.dram_tensor("oy", [B * C, F * F, PATCH], bf16, kind="ExternalOutput")

    from contextlib import ExitStack

    with ExitStack() as ctx:
        px = ctx.enter_context(nc.sbuf_tensor("px", [B * C, PR, PW], bf16))
        py = ctx.enter_context(nc.sbuf_tensor("py", [B * C, PR, PW], bf16))
        stk = [
            ctx.enter_context(
                nc.sbuf_tensor(f"stk{m}", [B * C, F, ROWS, W], bf16)
            )
            for m in range(NBUF)
        ]
        xl = ctx.enter_context(nc.semaphore("xl"))
        yl = ctx.enter_context(nc.semaphore("yl"))
        xlb = ctx.enter_context(nc.semaphore("xlb"))
        ylb = ctx.enter_context(nc.semaphore("ylb"))
        xc = ctx.enter_context(nc.semaphore("xc"))
        yc = ctx.enter_context(nc.semaphore("yc"))
        xst = ctx.enter_context(nc.semaphore("xst"))
        yst = ctx.enter_context(nc.semaphore("yst"))
        block = ctx.enter_context(nc.Block())

        # stacked-buffer fill order: m = 2*j + t (t=0 -> x, t=1 -> y)
        def emit_stores(eng, dst, copy_sem, store_sem, t):
            # j=0: three per-i stores, each gated on one DVE copy
            for i in range(F):
                eng.wait_ge(copy_sem, i + 1)
                eng.dma_start(
                    out=dst[:, i, :], in_=stk[t % NBUF][:, i]
                ).then_inc(store_sem, 16)
            # j=1,2: merged stacked stores
            for j in range(1, F):
                eng.wait_ge(copy_sem, F * (j + 1))
                m = 2 * j + t
                eng.dma_start(
                    out=dst[:, F * j : F * (j + 1), :],
                    in_=stk[m % NBUF][:],
                ).then_inc(store_sem, 16)
            eng.wait_ge(store_sem, 16 * (F + 2))

        # loads split at row ROWS: part A covers the i=0 window exactly,
        # so the first DVE copy (and the whole store stream) starts as
        # soon as A lands instead of waiting for the full padded slab.
        @block.sync
        def _(sync):
            sync.dma_start(out=px[:, :ROWS, :], in_=xs[:, :ROWS, :]).then_inc(
                xl, 16
            )
            sync.dma_start(out=px[:, ROWS:, :], in_=xs[:, ROWS:, :]).then_inc(
                xlb, 16
            )
            emit_stores(sync, ox, xc, xst, 0)

        @block.scalar
        def _(scalar):
            scalar.dma_start(
                out=py[:, :ROWS, :], in_=ys[:, :ROWS, :]
            ).then_inc(yl, 16)
            scalar.dma_start(
                out=py[:, ROWS:, :], in_=ys[:, ROWS:, :]
            ).then_inc(ylb, 16)
            emit_stores(scalar, oy, yc, yst, 1)

        @block.vector
        def _(vector):
            for m in range(2 * F):
                j, t = divmod(m, 2)
                tile = px if t == 0 else py
                load_sem = xl if t == 0 else yl
                load_b_sem = xlb if t == 0 else ylb
                copy_sem = xc if t == 0 else yc
                buf = stk[m % NBUF]
                if m >= NBUF:
                    # buffer reuse: all stores reading buf m-NBUF drained.
                    # m-NBUF is an x buffer iff (m-NBUF) even; j0 buffers
                    # (m-NBUF < 2) are consumed by 3 per-i stores, later
                    # ones by 1 merged store.
                    pm = m - NBUF
                    psem = xst if pm % 2 == 0 else yst
                    pj = pm // 2
                    cnt = F if pj == 0 else F + pj
                    vector.wait_ge(psem, 16 * cnt)
                for i in range(F):
                    if j == 0 and i == 0:
                        # i=0 window is rows [0:ROWS) -> load part A only
                        vector.wait_ge(load_sem, 16)
                    elif j == 0 and i == 1:
                        # i>=1 windows need the halo rows -> part B (FIFO
                        # on the ring, so B done implies A done)
                        vector.wait_ge(load_b_sem, 16)
                    vector.tensor_copy(
                        out=buf[:, i],
                        in_=tile[:, i * d : i * d + ROWS, j * d : j * d + W],
                    ).then_inc(copy_sem, 1)

    return nc


def kernel(inref_x: np.ndarray, inref_y: np.ndarray, dilation) -> tuple:
    d = int(dilation)
    x = np.asarray(inref_x, dtype=np.float32)
    y = np.asarray(inref_y, dtype=np.float32)

    if d not in _cache:
        _cache[d] = _build_nc(d)
    nc = _cache[d]

    px = np.pad(x, ((0, 0), (0, 0), (d, d), (d, d)), mode="reflect")
    py = np.pad(y, ((0, 0), (0, 0), (d, d), (d, d)), mode="reflect")
    pxb = _f32_to_bf16_u16(px)
    pyb = _f32_to_bf16_u16(py)
    PR = ROWS + 2 * d
    PW = W + 2 * d

    try:
        import ml_dtypes

        bf = np.dtype(ml_dtypes.bfloat16)
    except ImportError:
        bf = None

    in_maps = []
    for m in range(N_CORES):
        r0 = m * ROWS
        xs_u = np.ascontiguousarray(
            pxb[:, :, r0 : r0 + PR, :].reshape(B * C, PR, PW)
        )
        ys_u = np.ascontiguousarray(
            pyb[:, :, r0 : r0 + PR, :].reshape(B * C, PR, PW)
        )
        if bf is not None:
            xs_u = xs_u.view(bf)
            ys_u = ys_u.view(bf)
        in_maps.append({"xs": xs_u, "ys": ys_u})

    res = run_bass_kernel_spmd(nc, in_maps, core_ids=list(range(N_CORES)))

    def unpack(r, name):
        o = np.asarray(r[name])
        if o.dtype != np.uint16:
            o = o.view(np.uint16)
        # [B*C, 9(j-major: k_dev = 3j+i), PATCH] -> k_ref = 3i+j
        o = _bf16_u16_to_f32(o).reshape(B, C, F, F, ROWS, W)
        return np.ascontiguousarray(o.transpose(0, 3, 2, 1, 4, 5)).reshape(
            B, F * F * C, ROWS, W
        )

    agg_x = np.concatenate([unpack(r, "ox") for r in res.results], axis=2)
    agg_y = np.concatenate([unpack(r, "oy") for r in res.results], axis=2)
    return agg_x, agg_y
